# revision 19
# baseline (speedup 1.0000x reference)
"""BitLinear-1.58 (ternary-weight dense) Trainium2 kernel.

Reference computes:
    a  = clip(max(|x|, axis=-1), 1e-5)          [B,S,1]
    out = ((x / a) @ W.T) * (a * ws) + bias
The absmax normalization cancels algebraically -- (x/a)@W * a*ws == x@W * ws
exactly, including the clip (the same clipped `a` divides and multiplies).
So the kernel is a plain matmul + scale + bias:
    out = x @ W.T * ws + bias

Strategy (8 NeuronCores, tensor-parallel along out_features):
  - Each core owns N_C = 11008/8 = 1376 output features (column parallel).
  - x (8192 x 4096 fp32) is transposed on host to xT [K, M] and cast to
    fp16 (single pass).  Ternary weights are exact in fp16; only the x
    rounding contributes error (measured rel 2.1e-4 against the fp32
    reference, ~100x under the 2e-2 gate), so no hi/lo split pass is
    needed -- half the tensor-engine work of the 2-pass baseline.
  - Per output tile [128m x 512n]: 32 back-to-back fp16 PE matmuls
    (k-inner, one PSUM bank per accumulation group -- measured fastest;
    sharing the stationary across n-chunks forces PSUM-bank cycling,
    which costs far more than the hidden LDWEIGHTS) accumulate in PSUM;
    a single DVE scalar_tensor_tensor applies out = psum * ws + bias;
    DMA to DRAM in the natural [M, N_C] layout.
  - fp8e4 DoubleRow (FP8_KT > 0 routes leading k-tiles through 2-k-tile
    fp8 matmuls) was measured SLOWER per contracted element than fp16 on
    this backend (~206-278ns vs ~71-137ns per 512-col matmul) and is
    disabled; the plumbing is kept for reference.
"""

import numpy as np
import ml_dtypes

import concourse.bass as bass
import concourse.mybir as mybir
import concourse.tile as tile
from concourse import bacc
from concourse.bass_utils import run_bass_kernel_spmd

P = 128
B_DIM, S_DIM, K_DIM, N_FULL = 4, 2048, 4096, 11008
M_DIM = B_DIM * S_DIM            # 8192
N_CORES = 8
N_C = N_FULL // N_CORES          # 1376 per-core output features
KT = K_DIM // P                  # 32 k-tiles
M_BLK = 256                      # m columns per x slab
MT_PER_BLK = M_BLK // P          # stationary tiles per slab
N_CHUNKS = (512, 512, 352)       # moving-operand free-dim chunks (sum = N_C)
FP8_KT = 0                       # leading k-tiles in fp8 DoubleRow (even)
KT16 = KT - FP8_KT               # trailing k-tiles in fp16
MB_N = M_DIM // M_BLK            # number of m-blocks
BLOCKED_X = False                # store x pre-tiled per m-block (contiguous DMA)
OUT_F16 = False                  # write output as fp16 (host upcasts)

assert FP8_KT % 2 == 0


def build_nc(n_repeat=1):
    """n_repeat > 1 re-runs the whole computation that many times inside one
    NEFF (identical output) -- used only for overhead-free timing:
    hw_time = (t[R] - t[1]) / (R - 1)."""
    nc = bacc.Bacc("TRN2", target_bir_lowering=False, debug=False)
    f8, f16, f32 = mybir.dt.float8e4, mybir.dt.float16, mybir.dt.float32

    if FP8_KT:
        xt8 = nc.dram_tensor("xt8", [FP8_KT * P, M_DIM], f8, kind="ExternalInput")
        wt8 = nc.dram_tensor("wt8", [FP8_KT * P, N_C], f8, kind="ExternalInput")
        xt8_v = xt8.rearrange("(kt p) m -> p kt m", p=P)
        wt8_v = wt8.rearrange("(kt p) n -> p kt n", p=P)
    if KT16:
        if BLOCKED_X:
            xt16 = nc.dram_tensor("xt16", [MB_N, P, KT16, M_BLK], f16,
                                  kind="ExternalInput")
        else:
            xt16 = nc.dram_tensor("xt16", [KT16 * P, M_DIM], f16,
                                  kind="ExternalInput")
            xt16_v = xt16.rearrange("(kt p) m -> p kt m", p=P)
        wt16 = nc.dram_tensor("wt16", [KT16 * P, N_C], f16, kind="ExternalInput")
        wt16_v = wt16.rearrange("(kt p) n -> p kt n", p=P)
    bias_rep = nc.dram_tensor("bias_rep", [P, N_C], f32, kind="ExternalInput")
    ws_col = nc.dram_tensor("ws_col", [P, 1], f32, kind="ExternalInput")
    out_dt = f16 if OUT_F16 else f32
    out = nc.dram_tensor("out", [M_DIM, N_C], out_dt, kind="ExternalOutput")

    with tile.TileContext(nc) as tc:
        with tc.tile_pool(name="const", bufs=1) as const, \
             tc.tile_pool(name="xp", bufs=4) as xp, \
             tc.tile_pool(name="op", bufs=4) as op, \
             tc.tile_pool(name="ps", bufs=8, space="PSUM") as ps:
            # weights fully SBUF-resident: loaded once, reused by all m-blocks
            if FP8_KT:
                w8_sb = const.tile([P, FP8_KT, N_C], f8)
                nc.sync.dma_start(w8_sb[:], wt8_v[:])
            if KT16:
                w16_sb = const.tile([P, KT16, N_C], f16)
                nc.sync.dma_start(w16_sb[:], wt16_v[:])
            bias_sb = const.tile([P, N_C], f32)
            nc.sync.dma_start(bias_sb[:], bias_rep[:])
            ws_sb = const.tile([P, 1], f32)
            nc.sync.dma_start(ws_sb[:], ws_col[:])

            for mb_rep in range(n_repeat * (M_DIM // M_BLK)):
                mb = mb_rep % (M_DIM // M_BLK)
                mo = mb * M_BLK
                if FP8_KT:
                    x8 = xp.tile([P, FP8_KT, M_BLK], f8, tag="x8")
                    nc.sync.dma_start(x8[:], xt8_v[:, :, mo:mo + M_BLK])
                if KT16:
                    x16 = xp.tile([P, KT16, M_BLK], f16, tag="x16")
                    if BLOCKED_X:
                        nc.sync.dma_start(x16[:], xt16[mb])
                    else:
                        nc.sync.dma_start(x16[:], xt16_v[:, :, mo:mo + M_BLK])
                no = 0
                for ncw in N_CHUNKS:
                    for mt in range(MT_PER_BLK):
                        mtile = slice(mt * P, (mt + 1) * P)
                        pt = ps.tile([P, 512], f32, name="pt", tag="pt")
                        for kp in range(0, FP8_KT, 2):
                            nc.tensor.matmul(
                                pt[:, :ncw], x8[:, kp:kp + 2, mtile],
                                w8_sb[:, kp:kp + 2, no:no + ncw],
                                start=(kp == 0),
                                stop=(KT16 == 0 and kp == FP8_KT - 2),
                                perf_mode=mybir.MatmulPerfMode.DoubleRow)
                        for k in range(KT16):
                            nc.tensor.matmul(
                                pt[:, :ncw], x16[:, k, mtile],
                                w16_sb[:, k, no:no + ncw],
                                start=(FP8_KT == 0 and k == 0),
                                stop=(k == KT16 - 1))
                        ot = op.tile([P, 512], out_dt, tag="o")
                        nc.vector.scalar_tensor_tensor(
                            ot[:, :ncw], pt[:, :ncw], ws_sb[:, 0:1],
                            bias_sb[:, no:no + ncw],
                            op0=mybir.AluOpType.mult, op1=mybir.AluOpType.add)
                        nc.sync.dma_start(
                            out[mo + mt * P:mo + (mt + 1) * P, no:no + ncw],
                            ot[:, :ncw])
                    no += ncw

    nc.compile()
    return nc


def prep_inputs(x, weight_ternary, weight_scale, bias):
    x2d = np.asarray(x, dtype=np.float32).reshape(M_DIM, K_DIM)
    xt = np.ascontiguousarray(x2d.T)                      # [K, M] fp32
    K8 = FP8_KT * P
    ws_col = np.full((P, 1), np.float32(np.asarray(weight_scale).reshape(-1)[0]),
                     dtype=np.float32)
    xt8 = np.ascontiguousarray(xt[:K8]).astype(ml_dtypes.float8_e4m3)
    xt16 = np.ascontiguousarray(xt[K8:]).astype(np.float16)
    if BLOCKED_X and KT16:
        # [mb, p, kt*m_blk]: per-partition-contiguous slab per m-block
        xt16 = np.ascontiguousarray(
            xt16.reshape(KT16, P, MB_N, M_BLK)
            .transpose(2, 1, 0, 3)
            .reshape(MB_N, P, KT16, M_BLK))
    w_all = np.asarray(weight_ternary)
    in_maps = []
    for c in range(N_CORES):
        rows = slice(c * N_C, (c + 1) * N_C)
        wt_c = np.ascontiguousarray(w_all[rows, :].T)     # [K, N_C] int8
        bias_c = np.ascontiguousarray(
            np.broadcast_to(np.asarray(bias, dtype=np.float32)[rows][None, :],
                            (P, N_C)))
        m = {"bias_rep": bias_c, "ws_col": ws_col}
        if FP8_KT:
            m["xt8"] = xt8
            m["wt8"] = wt_c[:K8].astype(ml_dtypes.float8_e4m3)
        if KT16:
            m["xt16"] = xt16
            m["wt16"] = wt_c[K8:].astype(np.float16)
        in_maps.append(m)
    return in_maps


def gather_output(results):
    cols = [results[c]["out"].astype(np.float32) for c in range(N_CORES)]
    return np.concatenate(cols, axis=1).reshape(B_DIM, S_DIM, N_FULL)


def kernel(x, weight_ternary, weight_scale, bias):
    nc = build_nc()
    in_maps = prep_inputs(x, weight_ternary, weight_scale, bias)
    res = run_bass_kernel_spmd(nc, in_maps, core_ids=list(range(N_CORES)))
    return gather_output(res.results)


if __name__ == "__main__":
    rng = np.random.default_rng(0)
    x = rng.standard_normal((B_DIM, S_DIM, K_DIM)).astype(np.float32)
    w = rng.integers(-1, 2, size=(N_FULL, K_DIM)).astype(np.int8)
    ws = np.full((1,), 0.02, np.float32)
    b = (rng.standard_normal(N_FULL) * 0.01).astype(np.float32)
    out = kernel(x, w, ws, b)
    print(out.shape, out.dtype)
